# revision 2
# baseline (speedup 1.0000x reference)
"""Trainium2 Bass kernel for a 2-layer GCN (CascadePredictionModel).

Model (per reference):
    src/dst = edge_index + self loops; deg over dst; norm_e = rsqrt(deg[src])*rsqrt(deg[dst])
    gcn(h, W, b) = segment_sum(norm * (h@W)[src], dst) + b
    h1 = relu(gcn(x,  W1, b1))
    h2 = relu(gcn(h1, W2, b2))
    pred = noise @ W3 + b3
    out = concat([h2, pred])            # [N+M, C]

Distribution strategy (8 NeuronCores, SPMD single NEFF):
  - Destination nodes are 1D-partitioned: core k owns dst rows [1250k, 1250k+1250).
  - Norm factorization: norm_e = dis[src]*dis[dst] is folded OUT of the
    selection matrices.  S holds edge MULTIPLICITIES (small exact ints) in
    fp8e4, halving its input size; dis[src] is folded into the z-rows at
    emission (activation scale) and dis[dst] + bias at the relu via a
    rank-1 (1/dis)(x)b matmul.
  - Feature matmul Z = h@W computed per-core for owned rows (weights
    replicated), scaled by dis, cast to fp16, AllGather'ed into a full
    [10000, 512] fp16 DRAM tensor.
  - Aggregation per dst tile of 128: dma_gather pulls the (per-tile deduped,
    src-sorted) source rows into SBUF as [128, nchunk, 512]; PE accumulates
    psum += S_c^T @ G_c over chunks (mixed fp8 lhsT x fp16 rhs).
  - Own-shard sources are gathered from the local pre-AllGather buffer so
    that ~1/5 of the gather volume overlaps the AllGather itself.
  - h1^T for the layer-2 matmul is built per-tile with PE transposes right
    after the layer-1 relu, so layer-2 feature matmuls pipeline behind the
    layer-1 aggregation.
  - pred rows are sharded 250/core and run inside the AllGather bubbles.

Per-exec input bytes are the dominant non-compute cost on this runtime
(host read-through ~21-45 GB/s), so inputs are minimized: S fp8, idx
tables shipped [16, n] and partition-replicated on device, bias as [1, .]
rows.  All gather traffic reads Internal DRAM (device-local, fast).
"""

import math
import time
from contextlib import ExitStack

import numpy as np
from ml_dtypes import float8_e4m3fn as f8e4

N, E, C, MPRED = 10000, 160000, 512, 2000
P = 8                 # cores
NPC = N // P          # 1250 nodes per core
TPB = 128             # dst-tile width
NT = (NPC + TPB - 1) // TPB   # 10 tiles / core (last has 98 dsts)
NPAD = NT * TPB       # 1280
PRED_PC = MPRED // P  # 250 pred rows per core
KT = C // 128         # 4 contraction tiles

_prog_cache: dict[int, tuple] = {}
LAST_RESULTS = None  # BassKernelResults of the most recent run (for test.py)


# ---------------------------------------------------------------- host tables
def _host_tables(edge_index):
    """Build per-core gather indices + multiplicity selection matrices.

    Returns (NCHUNKS, idxs_list, S_list, dis):
      NCHUNKS      : per-dst-tile (own, other) chunk counts (max over cores)
      idxs_list[k] : [16, NIDX//16] int16  (16-partition wrap; device x8)
      S_list[k]    : [128, sum(NCHUNKS), 128] fp8e4,
                     S[p, coff[t]+c, m] = multiplicity of edges
                     (src=u_t[c*128+p] -> dst=k*NPC+t*128+m)
      dis          : [N] float32 rsqrt(deg)
    """
    ei = np.asarray(edge_index).astype(np.int64)
    src = np.concatenate([ei[0], np.arange(N, dtype=np.int64)])
    dst = np.concatenate([ei[1], np.arange(N, dtype=np.int64)])
    deg = np.bincount(dst, minlength=N).astype(np.float64)
    dis = np.where(deg > 0, 1.0 / np.sqrt(np.maximum(deg, 1.0)), 0.0)

    order = np.lexsort((src, dst))
    src_s, dst_s = src[order], dst[order]

    per_tile = []   # [(k, t, u_own_local, u_oth, es, dloc)]
    nown = [1] * NT
    noth = [1] * NT
    for k in range(P):
        klo, khi = k * NPC, (k + 1) * NPC
        for t in range(NT):
            lo = k * NPC + t * TPB
            hi = min(khi, lo + TPB)
            m0 = np.searchsorted(dst_s, lo)
            m1 = np.searchsorted(dst_s, hi)
            es = src_s[m0:m1]
            u = np.unique(es)
            own_mask = (u >= klo) & (u < khi)
            u_own, u_oth = u[own_mask], u[~own_mask]
            nown[t] = max(nown[t], (len(u_own) + 127) // 128)
            noth[t] = max(noth[t], (len(u_oth) + 127) // 128)
            per_tile.append((k, t, u_own, u_oth, es, dst_s[m0:m1] - lo))
    NOWN, NOTH = tuple(nown), tuple(noth)
    nch = [a + b for a, b in zip(nown, noth)]
    coff = np.concatenate([[0], np.cumsum(nch)])  # chunk offset per tile
    NIDX = int(coff[-1]) * 128

    idxs_list, S_list = [], []
    for k in range(P):
        idxs = np.zeros(NIDX, dtype=np.int64)
        S = np.zeros((NIDX, TPB), dtype=np.float32)
        for (kk, t, u_own, u_oth, es, dloc) in per_tile[k * NT:(k + 1) * NT]:
            base = int(coff[t]) * 128          # own group first
            obase = base + NOWN[t] * 128       # then other group
            idxs[base:base + len(u_own)] = u_own - k * NPC  # local rows in zb
            idxs[obase:obase + len(u_oth)] = u_oth          # global rows in zf
            # position of each edge's src within the tile's gathered rows
            own_e = (es >= k * NPC) & (es < (k + 1) * NPC)
            pos = np.empty(len(es), dtype=np.int64)
            pos[own_e] = base + np.searchsorted(u_own, es[own_e])
            pos[~own_e] = obase + np.searchsorted(u_oth, es[~own_e])
            np.add.at(S, (pos, dloc), 1.0)
        wrapped = np.ascontiguousarray(idxs.reshape(-1, 16).T).astype(np.int16)
        S_host = np.ascontiguousarray(
            S.reshape(int(coff[-1]), 128, TPB).transpose(1, 0, 2)
        ).astype(f8e4)
        idxs_list.append(wrapped)
        S_list.append(S_host)
    return (NOWN, NOTH), idxs_list, S_list, dis.astype(np.float32)


# ---------------------------------------------------------------- device prog
def _build_program(NCHUNKS, sim1core=False, loops=1, no_cc=False, no_gather=False,
                   nqueues=4, scratch=49152):
    """sim1core=True builds a single-core timing variant for TimelineSim:
    collectives are replaced by a DRAM->DRAM DMA of the same dependency
    shape, everything else identical.  loops>1 repeats the compute body."""
    import concourse.bacc as bacc
    import concourse.mybir as mybir
    import concourse.tile as tile

    f16, f32, i16 = mybir.dt.float16, mybir.dt.float32, mybir.dt.int16
    f8 = mybir.dt.float8e4
    Relu = mybir.ActivationFunctionType.Relu
    Copy = mybir.ActivationFunctionType.Copy
    NOWN, NOTH = NCHUNKS
    COFF = [0]
    for a, b in zip(NOWN, NOTH):
        COFF.append(COFF[-1] + a + b)
    NCTOT = COFF[-1]
    NIDX = NCTOT * 128

    nc = bacc.Bacc(
        "TRN2", target_bir_lowering=False, debug=False,
        num_devices=(1 if sim1core else P),
        num_swdge_queues=nqueues,
        dynamic_dma_scratch_size=scratch,
    )

    # xT split into NT column chunks so feature matmuls start after ~128KB
    xT_d = nc.dram_tensor("xT", [128, KT, NPAD], f16, kind="ExternalInput")
    w1_d = nc.dram_tensor("W1t", [128, KT, C], f16, kind="ExternalInput")
    w2_d = nc.dram_tensor("W2t", [128, KT, C], f16, kind="ExternalInput")
    w3_d = nc.dram_tensor("W3t", [128, KT, C], f16, kind="ExternalInput")
    s_d = nc.dram_tensor("S", [128, NCTOT, 128], f8, kind="ExternalInput")
    idx_d = nc.dram_tensor("idxs", [16, NIDX // 16], i16, kind="ExternalInput")
    # rows: [0]=b1, [1]=b2, [2]=b3 (fp16)
    bias_d = nc.dram_tensor("biasr", [1, 3, C], f16, kind="ExternalInput")
    # [1, NPAD] 1/dis of own rows, then [1, 128] ones (for pred bias)
    invdis_d = nc.dram_tensor("invdis", [1, NPAD + 128], f16, kind="ExternalInput")
    dis_d = nc.dram_tensor("dis", [128, NT], f32, kind="ExternalInput")
    ident_d = nc.dram_tensor("ident", [128, 128], f16, kind="ExternalInput")
    nzT_d = nc.dram_tensor("noiseT", [128, KT, 256], f16, kind="ExternalInput")
    out_d = nc.dram_tensor("out", [NPC + PRED_PC, C], f16, kind="ExternalOutput")

    zb = [nc.dram_tensor(f"zb{l}", [NPC, C], f16, kind="Internal") for l in range(2)]
    zf = [
        nc.dram_tensor(f"zf{l}", [N, C], f16, kind="Internal",
                       addr_space=("Local" if sim1core else "Shared"))
        for l in range(2)
    ]

    with tile.TileContext(nc) as tc, ExitStack() as ctx:
        consts = ctx.enter_context(tc.tile_pool(name="consts", bufs=1))
        zpool = ctx.enter_context(tc.tile_pool(name="z", bufs=6))
        gpool = ctx.enter_context(tc.tile_pool(name="g", bufs=6))
        gown = ctx.enter_context(tc.tile_pool(name="gown", bufs=NT))
        hpool = ctx.enter_context(tc.tile_pool(name="h", bufs=3))
        opool = ctx.enter_context(tc.tile_pool(name="o", bufs=3))
        fpsum = ctx.enter_context(tc.tile_pool(name="fps", bufs=3, space="PSUM"))
        apsum = ctx.enter_context(tc.tile_pool(name="aps", bufs=3, space="PSUM"))
        tpsum = ctx.enter_context(tc.tile_pool(name="tps", bufs=2, space="PSUM"))

        xT = consts.tile([128, KT, NPAD], f16, tag="xT")
        W1 = consts.tile([128, KT, C], f16, tag="W1")
        W2 = consts.tile([128, KT, C], f16, tag="W2")
        W3 = consts.tile([128, KT, C], f16, tag="W3")
        St = consts.tile([128, NCTOT, 128], f8, tag="S")
        idxt = consts.tile([128, NIDX // 16], i16, tag="idx")
        biast = consts.tile([1, 3, C], f16, tag="bias")
        invdis = consts.tile([1, NPAD + 128], f16, tag="invdis")
        dist = consts.tile([128, NT], f32, tag="dis")
        ident = consts.tile([128, 128], f16, tag="ident")
        nzT = consts.tile([128, KT, 256], f16, tag="nzT")
        h1T = consts.tile([128, KT, NPAD], f16, tag="h1T")

        # layer-1 feature operands first, fine-grained so the PE can start
        # after the first ~0.6MB of host read-through; S/idxs aren't needed
        # until after the first zb writes, so their loads trail.
        nc.sync.dma_start(W1[:], w1_d[:])
        for nt in range(NT):
            nc.sync.dma_start(xT[:, :, nt * 128:(nt + 1) * 128],
                              xT_d[:, :, nt * 128:(nt + 1) * 128])
        nc.sync.dma_start(dist[:], dis_d[:])
        nc.sync.dma_start(ident[:], ident_d[:])
        nc.sync.dma_start(invdis[:], invdis_d[:])
        nc.sync.dma_start(biast[:], bias_d[:])
        nc.sync.dma_start(idxt[0:16, :], idx_d[:])
        # replicate idx rows 16 -> 128 partitions (gather HW reads all 128)
        nc.sync.dma_start(idxt[16:32, :], idxt[0:16, :])
        nc.sync.dma_start(idxt[32:64, :], idxt[0:32, :])
        nc.sync.dma_start(idxt[64:128, :], idxt[0:64, :])
        SSEG = 4
        sb = [0] + [(NCTOT * (i + 1)) // SSEG for i in range(SSEG)]
        for i in range(SSEG):
            nc.sync.dma_start(St[:, sb[i]:sb[i + 1], :], s_d[:, sb[i]:sb[i + 1], :])
        nc.sync.dma_start(nzT[:], nzT_d[:])
        nc.sync.dma_start(W3[:], w3_d[:])
        nc.sync.dma_start(W2[:], w2_d[:])

        def feature_layer(lhsT, Wt, zb_d):
            # zb = dis (.) (h @ W)   (src-side norm folded in at emission)
            for nt in range(NT):
                ps = fpsum.tile([128, C], f32, tag="fps")
                for g in range(KT):
                    nc.tensor.matmul(
                        ps[:],
                        lhsT[:, g, nt * 128:(nt + 1) * 128],
                        Wt[:, g, :],
                        start=(g == 0),
                        stop=(g == KT - 1),
                    )
                zt = zpool.tile([128, C], f16, tag="z")
                nc.scalar.activation(zt[:], ps[:], Copy, scale=dist[:, nt:nt + 1])
                w = NPC - nt * 128 if nt == NT - 1 else 128
                nc.sync.dma_start(zb_d[nt * 128: nt * 128 + w, :], zt[:w, :])

        # gather groups: <=8 chunks (1024 idxs) per dma_gather so
        # single_packet fits and gather/matmul pipelining stays fine
        def tile_groups(nch):
            ha = min((nch + 1) // 2, 8)
            return [(0, ha), (ha, nch)] if nch > ha else [(0, nch)]

        _qn = [0]

        def gather_group(src_d, t, c0, c1, pool=None, tag="g"):
            """One dma_gather of chunks [c0,c1) of tile t from src_d."""
            qn = _qn[0] % nqueues
            _qn[0] += 1
            nch = c1 - c0
            G = (pool or gpool).tile([128, nch, C], f16, tag=tag)
            if no_gather:
                nc.vector.memset(G[:, 0, 0:16], 0.0)
            else:
                nc.gpsimd.dma_gather(
                    G[:],
                    src_d[:],
                    idxt[:, COFF[t] * 8 + c0 * 8: COFF[t] * 8 + c1 * 8],
                    nch * 128,
                    nch * 128,
                    C,
                    single_packet=(nch * 128 <= 1024),
                    queue_num=qn,
                )
            return G

        def agg_own_gathers(zb_d):
            """Own-shard gathers (from the local pre-AllGather buffer) — these
            only need zb, so they run during the AllGather wait."""
            return [gather_group(zb_d, t, 0, NOWN[t], pool=gown, tag="go")
                    for t in range(NT)]

        def agg_layer(lidx, zf_d, own_G, emit_out):
            # psum = sum_s mult * (dis.z)_s ; + rank-1 (1/dis) (x) b ;
            # emit applies relu/copy with scale=dis[dst]
            for t in range(NT):
                ps = apsum.tile([128, C], f32, tag="aps")
                for c in range(NOWN[t]):
                    nc.tensor.matmul(
                        ps[:], St[:, COFF[t] + c, :], own_G[t][:, c, :],
                        start=(c == 0), stop=False,
                    )
                for (c0, c1) in tile_groups(NOTH[t]):
                    G = gather_group(zf_d, t, NOWN[t] + c0, NOWN[t] + c1)
                    for c in range(c0, c1):
                        nc.tensor.matmul(
                            ps[:],
                            St[:, COFF[t] + NOWN[t] + c, :],
                            G[:, c - c0, :],
                            start=False,
                            stop=False,
                        )
                nc.tensor.matmul(
                    ps[:], invdis[:, t * 128:(t + 1) * 128], biast[:, lidx, :],
                    start=False, stop=True,
                )
                emit_out(t, ps)

        rg = [list(range(P))]

        def allgather(l):
            if no_cc or sim1core:
                nc.sync.dma_start(zf[l][:NPC, :], zb[l][:])
            else:
                nc.gpsimd.collective_compute(
                    "AllGather",
                    bacc.mybir.AluOpType.bypass,
                    replica_groups=rg,
                    ins=[zb[l][:]],
                    outs=[zf[l][:]],
                )

        # pred = noise @ W3 + b3 (no relu), 250 rows/core — one tile emitted in
        # each AllGather bubble so the PE has work while waiting.
        def pred_tile(mt):
            ps = fpsum.tile([128, C], f32, tag="fps")
            for g in range(KT):
                nc.tensor.matmul(
                    ps[:],
                    nzT[:, g, mt * 128:(mt + 1) * 128],
                    W3[:, g, :],
                    start=(g == 0),
                    stop=False,
                )
            nc.tensor.matmul(
                ps[:], invdis[:, NPAD:NPAD + 128], biast[:, 2, :],
                start=False, stop=True,
            )
            ot = opool.tile([128, C], f16, tag="o")
            nc.scalar.activation(ot[:], ps[:], Copy)
            w = min(128, PRED_PC - mt * 128)
            nc.sync.dma_start(
                out_d[NPC + mt * 128: NPC + mt * 128 + w, :], ot[:w, :]
            )

        def l1_out(t, ps):
            # h1 = relu(dis*psum + b1); PE-transpose into h1T so the layer-2
            # feature matmul for this node tile can start at once.
            ht = hpool.tile([128, C], f16, tag="h")
            nc.scalar.activation(ht[:], ps[:], Relu, scale=dist[:, t:t + 1])
            for g in range(KT):
                pt = tpsum.tile([128, 128], f16, tag="tps")
                nc.tensor.transpose(pt[:], ht[:, g * 128:(g + 1) * 128], ident[:])
                nc.vector.tensor_copy(h1T[:, g, t * 128:(t + 1) * 128], pt[:])

        def l2_out(t, ps):
            ot = opool.tile([128, C], f16, tag="o")
            nc.scalar.activation(ot[:], ps[:], Relu, scale=dist[:, t:t + 1])
            w = NPC - t * 128 if t == NT - 1 else 128
            nc.sync.dma_start(out_d[t * 128: t * 128 + w, :], ot[:w, :])

        for _rep in range(loops):
            # ---- layer 1
            feature_layer(xT, W1, zb[0])
            own1 = agg_own_gathers(zb[0])
            pred_tile(0)
            allgather(0)
            agg_layer(0, zf[0], own1, l1_out)
            # ---- layer 2
            feature_layer(h1T, W2, zb[1])
            own2 = agg_own_gathers(zb[1])
            pred_tile(1)
            allgather(1)
            agg_layer(1, zf[1], own2, l2_out)

    nc.compile()
    return nc


def _get_program(NCHUNKS):
    if NCHUNKS not in _prog_cache:
        _prog_cache[NCHUNKS] = _build_program(NCHUNKS)
    return _prog_cache[NCHUNKS]


# ---------------------------------------------------------------- entry point
def _prepare(x, edge_index, W1, b1, W2, b2, W3, b3, noise, num_missing_nodes=None,
             **_ignored):
    """Host preprocessing: returns (nc, in_maps)."""
    x = np.asarray(x, dtype=np.float32)
    W1 = np.asarray(W1, dtype=np.float32)
    W2 = np.asarray(W2, dtype=np.float32)
    W3 = np.asarray(W3, dtype=np.float32)
    b1 = np.asarray(b1, dtype=np.float32)
    b2 = np.asarray(b2, dtype=np.float32)
    b3 = np.asarray(b3, dtype=np.float32)
    noise = np.asarray(noise, dtype=np.float32)

    NCHUNKS, idxs_list, S_list, dis = _host_tables(edge_index)
    nc = _get_program(NCHUNKS)

    def wtiles(W):
        # [512, 512] -> [128, KT, 512] fp16
        return np.ascontiguousarray(
            W.reshape(KT, 128, C).transpose(1, 0, 2)
        ).astype(np.float16)

    biasr = np.stack([b1, b2, b3])[None, :, :].astype(np.float16)
    identity = np.eye(128, dtype=np.float16)
    w1t, w2t, w3t = wtiles(W1), wtiles(W2), wtiles(W3)

    in_maps = []
    for k in range(P):
        xs = np.zeros((NPAD, C), dtype=np.float16)
        xs[:NPC] = x[k * NPC:(k + 1) * NPC].astype(np.float16)
        xT = np.ascontiguousarray(
            xs.T.reshape(KT, 128, NPAD).transpose(1, 0, 2)
        )
        nz = np.zeros((256, C), dtype=np.float16)
        nz[:PRED_PC] = noise[k * PRED_PC:(k + 1) * PRED_PC].astype(np.float16)
        nzT = np.ascontiguousarray(nz.T.reshape(KT, 128, 256).transpose(1, 0, 2))
        dk = np.ones(NPAD, dtype=np.float32)
        dk[:NPC] = dis[k * NPC:(k + 1) * NPC]
        dk = np.maximum(dk, 1e-6)   # pad rows; deg>=1 so real dis>0
        dis_t = np.ascontiguousarray(dk.reshape(NT, 128).T)   # [128, NT]
        invd = np.ones((1, NPAD + 128), dtype=np.float32)
        invd[0, :NPAD] = 1.0 / dk
        in_maps.append({
            "xT": xT,
            "W1t": w1t,
            "W2t": w2t,
            "W3t": w3t,
            "S": S_list[k],
            "idxs": idxs_list[k],
            "biasr": biasr,
            "invdis": invd.astype(np.float16),
            "dis": dis_t,
            "ident": identity,
            "noiseT": nzT,
        })

    return nc, in_maps


def _assemble(results):
    out = np.empty((N + MPRED, C), dtype=np.float32)
    for k in range(P):
        o = results[k]["out"].astype(np.float32)
        out[k * NPC:(k + 1) * NPC] = o[:NPC]
        out[N + k * PRED_PC: N + (k + 1) * PRED_PC] = o[NPC:NPC + PRED_PC]
    return out


def kernel(x, edge_index, W1, b1, W2, b2, W3, b3, noise, num_missing_nodes=None,
           **_ignored):
    from concourse.bass_utils import run_bass_kernel_spmd

    nc, in_maps = _prepare(x, edge_index, W1, b1, W2, b2, W3, b3, noise,
                           num_missing_nodes)
    res = run_bass_kernel_spmd(nc, in_maps, core_ids=list(range(P)))
    global LAST_RESULTS
    LAST_RESULTS = res
    return _assemble(res.results)


if __name__ == "__main__":
    t0 = time.time()
    rng = np.random.default_rng(0)
    inputs = {
        "x": rng.standard_normal((N, C), dtype=np.float32),
        "edge_index": rng.integers(0, N, (2, E)).astype(np.int32),
        "W1": rng.standard_normal((C, C), dtype=np.float32) * 0.05,
        "b1": np.zeros(C, np.float32),
        "W2": rng.standard_normal((C, C), dtype=np.float32) * 0.05,
        "b2": np.zeros(C, np.float32),
        "W3": rng.standard_normal((C, C), dtype=np.float32) * 0.05,
        "b3": np.zeros(C, np.float32),
        "noise": rng.standard_normal((MPRED, C), dtype=np.float32),
        "num_missing_nodes": MPRED,
    }
    out = kernel(**inputs)
    print("kernel done", out.shape, time.time() - t0, "s")


# revision 3
# speedup vs baseline: 2.3567x; 2.3567x over previous
"""Trainium2 Bass kernel for a 2-layer GCN (CascadePredictionModel).

Model (per reference):
    src/dst = edge_index + self loops; deg over dst; norm_e = rsqrt(deg[src])*rsqrt(deg[dst])
    gcn(h, W, b) = segment_sum(norm * (h@W)[src], dst) + b
    h1 = relu(gcn(x,  W1, b1))
    h2 = relu(gcn(h1, W2, b2))
    pred = noise @ W3 + b3
    out = concat([h2, pred])            # [N+M, C]

Distribution strategy (8 NeuronCores, SPMD single NEFF):
  - Destination nodes are 1D-partitioned: core k owns dst rows [1250k, 1250k+1250).
  - Norm factorization: norm_e = dis[src]*dis[dst] is folded OUT of the
    selection matrices.  S holds edge MULTIPLICITIES (small exact ints) in
    fp8e4 (halving its bytes, exact); dis[src] is folded into the z-rows at
    emission (activation scale) and dis[dst] + bias at the relu via a
    rank-1 (1/dis)(x)b matmul.  Mixed fp8-lhsT x fp16-rhs matmuls are exact
    on TRN2 (verified on hw).
  - Feature matmul Z = h@W computed per-core for owned rows (weights
    replicated), scaled by dis, cast to fp16, AllGather'ed into a full
    [10000, 512] fp16 DRAM tensor.
  - Aggregation per dst tile of 128: dma_gather pulls the (per-tile deduped,
    src-sorted) source rows into SBUF as [128, nchunk, 512]; PE accumulates
    psum += S_c^T @ G_c over chunks.
  - Own-shard sources are gathered from the local pre-AllGather buffer so
    that ~1/5 of the gather volume overlaps the AllGather itself.
  - h1^T for the layer-2 matmul is built per-tile with PE transposes right
    after the layer-1 relu, so layer-2 feature matmuls pipeline behind the
    layer-1 aggregation.
  - pred rows are sharded 250/core and run inside the AllGather bubbles.

Runtime quirk this layout targets: each ExternalInput TENSOR costs ~40us
per exec on this PJRT path (host read-through), almost independent of
size — so ALL inputs are packed into a single [128, F] fp16 "blob" tensor
per core, DMA'd in 4 pipelined pieces into 4 SBUF tiles; fp8/int16/fp32
regions are accessed via bitcast views.  Gather traffic reads Internal
DRAM (device-local, fast) and is unaffected.
"""

import time
from contextlib import ExitStack

import numpy as np
from ml_dtypes import float8_e4m3fn as f8e4

N, E, C, MPRED = 10000, 160000, 512, 2000
P = 8                 # cores
NPC = N // P          # 1250 nodes per core
TPB = 128             # dst-tile width
NT = (NPC + TPB - 1) // TPB   # 10 tiles / core (last has 98 dsts)
NPAD = NT * TPB       # 1280
PRED_PC = MPRED // P  # 250 pred rows per core
KT = C // 128         # 4 contraction tiles

_prog_cache: dict[int, tuple] = {}
LAST_RESULTS = None  # BassKernelResults of the most recent run (for test.py)


# ------------------------------------------------------------- blob offsets
def _blob_layout(NCTOT):
    """f16 free-element offsets of each region; 4 DMA pieces (A, B2, B3, B4)."""
    NIDX16 = NCTOT * 8          # NIDX // 16
    A = dict(W1=0, xT=KT * C, dis=KT * C + KT * NPAD)
    A["ident"] = A["dis"] + 2 * NT            # dis is f32: 2*NT f16 elems
    AF = A["ident"] + 128
    B2 = dict(invdis=0, bias=NPAD + 128)
    B2["idx"] = B2["bias"] + 3 * C
    B2F = B2["idx"] + NIDX16
    B3F = NCTOT * 64                          # S fp8: NCTOT*128 bytes
    B4 = dict(nzT=0, W3=KT * 256, W2=KT * 256 + KT * C)
    B4F = B4["W2"] + KT * C
    return A, AF, B2, B2F, B3F, B4, B4F


# ---------------------------------------------------------------- host tables
def _host_tables(edge_index):
    """Build per-core gather indices + multiplicity selection matrices.

    Returns (NCHUNKS, idxs_list, S_list, dis):
      NCHUNKS      : per-dst-tile (own, other) chunk counts (max over cores)
      idxs_list[k] : [128, NIDX//16] int16  (16-partition wrap, tiled x8)
      S_list[k]    : [128, sum(NCHUNKS)*128] fp8e4 multiplicities
      dis          : [N] float32 rsqrt(deg)
    """
    ei = np.asarray(edge_index).astype(np.int64)
    src = np.concatenate([ei[0], np.arange(N, dtype=np.int64)])
    dst = np.concatenate([ei[1], np.arange(N, dtype=np.int64)])
    deg = np.bincount(dst, minlength=N).astype(np.float64)
    dis = np.where(deg > 0, 1.0 / np.sqrt(np.maximum(deg, 1.0)), 0.0)

    order = np.lexsort((src, dst))
    src_s, dst_s = src[order], dst[order]

    per_tile = []
    nown = [1] * NT
    noth = [1] * NT
    for k in range(P):
        klo, khi = k * NPC, (k + 1) * NPC
        for t in range(NT):
            lo = k * NPC + t * TPB
            hi = min(khi, lo + TPB)
            m0 = np.searchsorted(dst_s, lo)
            m1 = np.searchsorted(dst_s, hi)
            es = src_s[m0:m1]
            u = np.unique(es)
            own_mask = (u >= klo) & (u < khi)
            u_own, u_oth = u[own_mask], u[~own_mask]
            nown[t] = max(nown[t], (len(u_own) + 127) // 128)
            noth[t] = max(noth[t], (len(u_oth) + 127) // 128)
            per_tile.append((k, t, u_own, u_oth, es, dst_s[m0:m1] - lo))
    NOWN, NOTH = tuple(nown), tuple(noth)
    nch = [a + b for a, b in zip(nown, noth)]
    coff = np.concatenate([[0], np.cumsum(nch)])
    NIDX = int(coff[-1]) * 128

    idxs_list, S_list = [], []
    for k in range(P):
        idxs = np.zeros(NIDX, dtype=np.int64)
        S = np.zeros((NIDX, TPB), dtype=np.float32)
        for (kk, t, u_own, u_oth, es, dloc) in per_tile[k * NT:(k + 1) * NT]:
            base = int(coff[t]) * 128          # own group first
            obase = base + NOWN[t] * 128       # then other group
            idxs[base:base + len(u_own)] = u_own - k * NPC  # local rows in zb
            idxs[obase:obase + len(u_oth)] = u_oth          # global rows in zf
            own_e = (es >= k * NPC) & (es < (k + 1) * NPC)
            pos = np.empty(len(es), dtype=np.int64)
            pos[own_e] = base + np.searchsorted(u_own, es[own_e])
            pos[~own_e] = obase + np.searchsorted(u_oth, es[~own_e])
            np.add.at(S, (pos, dloc), 1.0)
        wrapped = np.tile(idxs.reshape(-1, 16).T, (8, 1)).astype(np.int16)
        # [NIDX, 128] -> [128(part), NCTOT, 128] -> flatten free
        S_host = np.ascontiguousarray(
            S.reshape(int(coff[-1]), 128, TPB).transpose(1, 0, 2)
        ).astype(f8e4).reshape(128, -1)
        idxs_list.append(np.ascontiguousarray(wrapped))
        S_list.append(S_host)
    return (NOWN, NOTH), idxs_list, S_list, dis.astype(np.float32)


# ---------------------------------------------------------------- device prog
def _build_program(NCHUNKS, sim1core=False, loops=1, no_cc=False, no_gather=False,
                   nqueues=4, scratch=49152):
    import concourse.bacc as bacc
    import concourse.mybir as mybir
    import concourse.tile as tile

    f16, f32, i16 = mybir.dt.float16, mybir.dt.float32, mybir.dt.int16
    f8 = mybir.dt.float8e4
    Relu = mybir.ActivationFunctionType.Relu
    Copy = mybir.ActivationFunctionType.Copy
    NOWN, NOTH = NCHUNKS
    COFF = [0]
    for a, b in zip(NOWN, NOTH):
        COFF.append(COFF[-1] + a + b)
    NCTOT = COFF[-1]
    A, AF, B2, B2F, B3F, B4, B4F = _blob_layout(NCTOT)
    TOTF = AF + B2F + B3F + B4F

    nc = bacc.Bacc(
        "TRN2", target_bir_lowering=False, debug=False,
        num_devices=(1 if sim1core else P),
        num_swdge_queues=nqueues,
        dynamic_dma_scratch_size=scratch,
    )

    blob_d = nc.dram_tensor("blob", [128, TOTF], f16, kind="ExternalInput")
    out_d = nc.dram_tensor("out", [NPC + PRED_PC, C], f16, kind="ExternalOutput")

    zb = [nc.dram_tensor(f"zb{l}", [NPC, C], f16, kind="Internal") for l in range(2)]
    zf = [
        nc.dram_tensor(f"zf{l}", [N, C], f16, kind="Internal",
                       addr_space=("Local" if sim1core else "Shared"))
        for l in range(2)
    ]

    with tile.TileContext(nc) as tc, ExitStack() as ctx:
        consts = ctx.enter_context(tc.tile_pool(name="consts", bufs=1))
        zpool = ctx.enter_context(tc.tile_pool(name="z", bufs=6))
        gpool = ctx.enter_context(tc.tile_pool(name="g", bufs=6))
        gown = ctx.enter_context(tc.tile_pool(name="gown", bufs=NT))
        hpool = ctx.enter_context(tc.tile_pool(name="h", bufs=3))
        opool = ctx.enter_context(tc.tile_pool(name="o", bufs=3))
        fpsum = ctx.enter_context(tc.tile_pool(name="fps", bufs=3, space="PSUM"))
        apsum = ctx.enter_context(tc.tile_pool(name="aps", bufs=3, space="PSUM"))
        tpsum = ctx.enter_context(tc.tile_pool(name="tps", bufs=2, space="PSUM"))

        tA = consts.tile([128, AF], f16, tag="tA")
        tB2 = consts.tile([128, B2F], f16, tag="tB2")
        tB3 = consts.tile([128, B3F], f16, tag="tB3")
        tB4 = consts.tile([128, B4F], f16, tag="tB4")
        h1T = consts.tile([128, KT, NPAD], f16, tag="h1T")

        # pipelined input pieces, ordered by first use
        o2, o3, o4 = AF, AF + B2F, AF + B2F + B3F
        nc.sync.dma_start(tA[:], blob_d[:, 0:AF])
        nc.sync.dma_start(tB2[:], blob_d[:, o2:o2 + B2F])
        nc.sync.dma_start(tB3[:], blob_d[:, o3:o3 + B3F])
        nc.sync.dma_start(tB4[:], blob_d[:, o4:o4 + B4F])

        # ---- views into the blob tiles
        def W1v(g):
            return tA[:, A["W1"] + g * C: A["W1"] + (g + 1) * C]

        def xTv(g, lo, w):
            o = A["xT"] + g * NPAD + lo
            return tA[:, o:o + w]

        dis_f32 = tA[:, A["dis"]:A["dis"] + 2 * NT].bitcast(f32)  # [128, NT]
        ident = tA[:, A["ident"]:A["ident"] + 128]

        def invdisv(t):   # t in 0..NT-1; t=NT selects the ones row (pred bias)
            o = B2["invdis"] + t * 128
            return tB2[0:1, o:o + 128]

        def biasv(l):
            o = B2["bias"] + l * C
            return tB2[0:1, o:o + C]

        def idxv(e0, e1):  # i16 element range
            return tB2[:, B2["idx"] + e0:B2["idx"] + e1].bitcast(i16)

        def Sv(c):        # chunk c -> [128, 128] fp8
            return tB3[:, c * 64:(c + 1) * 64].bitcast(f8)

        def nzTv(g, mt):
            o = B4["nzT"] + g * 256 + mt * 128
            return tB4[:, o:o + 128]

        def W2v(g):
            return tB4[:, B4["W2"] + g * C: B4["W2"] + (g + 1) * C]

        def W3v(g):
            return tB4[:, B4["W3"] + g * C: B4["W3"] + (g + 1) * C]

        def feature_layer(lhsT_v, Wv, zb_d):
            # zb = dis (.) (h @ W)   (src-side norm folded in at emission)
            for nt in range(NT):
                ps = fpsum.tile([128, C], f32, tag="fps")
                for g in range(KT):
                    nc.tensor.matmul(
                        ps[:], lhsT_v(g, nt * 128, 128), Wv(g),
                        start=(g == 0), stop=(g == KT - 1),
                    )
                zt = zpool.tile([128, C], f16, tag="z")
                nc.scalar.activation(zt[:], ps[:], Copy,
                                     scale=dis_f32[:, nt:nt + 1])
                w = NPC - nt * 128 if nt == NT - 1 else 128
                nc.sync.dma_start(zb_d[nt * 128: nt * 128 + w, :], zt[:w, :])

        def tile_groups(nch):
            ha = min((nch + 1) // 2, 8)
            return [(0, ha), (ha, nch)] if nch > ha else [(0, nch)]

        _qn = [0]

        def gather_group(src_d, t, c0, c1, pool=None, tag="g"):
            """One dma_gather of chunks [c0,c1) of tile t from src_d."""
            qn = _qn[0] % nqueues
            _qn[0] += 1
            nch = c1 - c0
            G = (pool or gpool).tile([128, nch, C], f16, tag=tag)
            if no_gather:
                nc.vector.memset(G[:, 0, 0:16], 0.0)
            else:
                nc.gpsimd.dma_gather(
                    G[:],
                    src_d[:],
                    idxv(COFF[t] * 8 + c0 * 8, COFF[t] * 8 + c1 * 8),
                    nch * 128,
                    nch * 128,
                    C,
                    single_packet=(nch * 128 <= 1024),
                    queue_num=qn,
                )
            return G

        def agg_own_gathers(zb_d):
            """Own-shard gathers (from the local pre-AllGather buffer) — these
            only need zb, so they run during the AllGather wait."""
            return [gather_group(zb_d, t, 0, NOWN[t], pool=gown, tag="go")
                    for t in range(NT)]

        def agg_layer(lidx, zf_d, own_G, emit_out):
            # psum = sum_s mult * (dis.z)_s ; + rank-1 (1/dis)(x)b
            for t in range(NT):
                ps = apsum.tile([128, C], f32, tag="aps")
                for c in range(NOWN[t]):
                    nc.tensor.matmul(
                        ps[:], Sv(COFF[t] + c), own_G[t][:, c, :],
                        start=(c == 0), stop=False,
                    )
                for (c0, c1) in tile_groups(NOTH[t]):
                    G = gather_group(zf_d, t, NOWN[t] + c0, NOWN[t] + c1)
                    for c in range(c0, c1):
                        nc.tensor.matmul(
                            ps[:], Sv(COFF[t] + NOWN[t] + c), G[:, c - c0, :],
                            start=False, stop=False,
                        )
                nc.tensor.matmul(
                    ps[:], invdisv(t), biasv(lidx), start=False, stop=True,
                )
                emit_out(t, ps)

        rg = [list(range(P))]

        def allgather(l):
            if no_cc or sim1core:
                nc.sync.dma_start(zf[l][:NPC, :], zb[l][:])
            else:
                nc.gpsimd.collective_compute(
                    "AllGather",
                    bacc.mybir.AluOpType.bypass,
                    replica_groups=rg,
                    ins=[zb[l][:]],
                    outs=[zf[l][:]],
                )

        # pred = noise @ W3 + b3 (no relu), 250 rows/core — one tile emitted in
        # each AllGather bubble so the PE has work while waiting.
        def pred_tile(mt):
            ps = fpsum.tile([128, C], f32, tag="fps")
            for g in range(KT):
                nc.tensor.matmul(
                    ps[:], nzTv(g, mt), W3v(g), start=(g == 0), stop=False,
                )
            nc.tensor.matmul(
                ps[:], invdisv(NT), biasv(2), start=False, stop=True,
            )
            ot = opool.tile([128, C], f16, tag="o")
            nc.scalar.activation(ot[:], ps[:], Copy)
            w = min(128, PRED_PC - mt * 128)
            nc.sync.dma_start(
                out_d[NPC + mt * 128: NPC + mt * 128 + w, :], ot[:w, :]
            )

        def l1_out(t, ps):
            # h1 = relu(dis*psum + b1); PE-transpose into h1T so the layer-2
            # feature matmul for this node tile can start at once.
            ht = hpool.tile([128, C], f16, tag="h")
            nc.scalar.activation(ht[:], ps[:], Relu, scale=dis_f32[:, t:t + 1])
            for g in range(KT):
                pt = tpsum.tile([128, 128], f16, tag="tps")
                nc.tensor.transpose(pt[:], ht[:, g * 128:(g + 1) * 128], ident)
                nc.vector.tensor_copy(h1T[:, g, t * 128:(t + 1) * 128], pt[:])

        def l2_out(t, ps):
            ot = opool.tile([128, C], f16, tag="o")
            nc.scalar.activation(ot[:], ps[:], Relu, scale=dis_f32[:, t:t + 1])
            w = NPC - t * 128 if t == NT - 1 else 128
            nc.sync.dma_start(out_d[t * 128: t * 128 + w, :], ot[:w, :])

        def h1Tv(g, lo, w):
            return h1T[:, g, lo:lo + w]

        for _rep in range(loops):
            # ---- layer 1
            feature_layer(xTv, W1v, zb[0])
            own1 = agg_own_gathers(zb[0])
            pred_tile(0)
            allgather(0)
            agg_layer(0, zf[0], own1, l1_out)
            # ---- layer 2
            feature_layer(h1Tv, W2v, zb[1])
            own2 = agg_own_gathers(zb[1])
            pred_tile(1)
            allgather(1)
            agg_layer(1, zf[1], own2, l2_out)

    nc.compile()
    return nc


def _get_program(NCHUNKS):
    if NCHUNKS not in _prog_cache:
        _prog_cache[NCHUNKS] = _build_program(NCHUNKS)
    return _prog_cache[NCHUNKS]


# ---------------------------------------------------------------- entry point
def _prepare(x, edge_index, W1, b1, W2, b2, W3, b3, noise, num_missing_nodes=None,
             **_ignored):
    """Host preprocessing: returns (nc, in_maps)."""
    x = np.asarray(x, dtype=np.float32)
    W1 = np.asarray(W1, dtype=np.float32)
    W2 = np.asarray(W2, dtype=np.float32)
    W3 = np.asarray(W3, dtype=np.float32)
    b1 = np.asarray(b1, dtype=np.float32)
    b2 = np.asarray(b2, dtype=np.float32)
    b3 = np.asarray(b3, dtype=np.float32)
    noise = np.asarray(noise, dtype=np.float32)

    NCHUNKS, idxs_list, S_list, dis = _host_tables(edge_index)
    nc = _get_program(NCHUNKS)
    NOWN, NOTH = NCHUNKS
    NCTOT = sum(NOWN) + sum(NOTH)
    A, AF, B2, B2F, B3F, B4, B4F = _blob_layout(NCTOT)
    TOTF = AF + B2F + B3F + B4F
    o2, o3, o4 = AF, AF + B2F, AF + B2F + B3F

    def wtiles(W):
        # [512, 512] -> [128, KT*512] fp16
        return np.ascontiguousarray(
            W.reshape(KT, 128, C).transpose(1, 0, 2)
        ).astype(np.float16).reshape(128, -1)

    biasr = np.concatenate([b1, b2, b3]).astype(np.float16)      # [3*C]
    identity = np.eye(128, dtype=np.float16)
    w1t, w2t, w3t = wtiles(W1), wtiles(W2), wtiles(W3)

    in_maps = []
    for k in range(P):
        blob = np.zeros((128, TOTF), dtype=np.float16)
        # ---- A piece
        blob[:, A["W1"]:A["W1"] + KT * C] = w1t
        xs = np.zeros((NPAD, C), dtype=np.float16)
        xs[:NPC] = x[k * NPC:(k + 1) * NPC].astype(np.float16)
        xT = np.ascontiguousarray(
            xs.T.reshape(KT, 128, NPAD).transpose(1, 0, 2)
        ).reshape(128, -1)
        blob[:, A["xT"]:A["xT"] + KT * NPAD] = xT
        dk = np.ones(NPAD, dtype=np.float32)
        dk[:NPC] = dis[k * NPC:(k + 1) * NPC]
        dk = np.maximum(dk, 1e-6)
        dis_t = np.ascontiguousarray(dk.reshape(NT, 128).T)      # [128, NT] f32
        blob[:, A["dis"]:A["dis"] + 2 * NT] = dis_t.view(np.float16)
        blob[:, A["ident"]:A["ident"] + 128] = identity
        # ---- B2 piece (invdis | bias | idx), invdis+bias replicated over rows
        invd = np.ones(NPAD + 128, dtype=np.float32)
        invd[:NPAD] = 1.0 / dk
        blob[:, o2 + B2["invdis"]:o2 + B2["invdis"] + NPAD + 128] = \
            invd.astype(np.float16)[None, :]
        blob[:, o2 + B2["bias"]:o2 + B2["bias"] + 3 * C] = biasr[None, :]
        blob[:, o2 + B2["idx"]:o2 + B2["idx"] + NCTOT * 8] = \
            idxs_list[k].view(np.float16)
        # ---- B3 piece (S fp8)
        blob[:, o3:o3 + B3F] = S_list[k].view(np.float16)
        # ---- B4 piece (nzT | W3 | W2)
        nz = np.zeros((256, C), dtype=np.float16)
        nz[:PRED_PC] = noise[k * PRED_PC:(k + 1) * PRED_PC].astype(np.float16)
        nzT = np.ascontiguousarray(
            nz.T.reshape(KT, 128, 256).transpose(1, 0, 2)).reshape(128, -1)
        blob[:, o4 + B4["nzT"]:o4 + B4["nzT"] + KT * 256] = nzT
        blob[:, o4 + B4["W3"]:o4 + B4["W3"] + KT * C] = w3t
        blob[:, o4 + B4["W2"]:o4 + B4["W2"] + KT * C] = w2t
        in_maps.append({"blob": blob})

    return nc, in_maps


def _assemble(results):
    out = np.empty((N + MPRED, C), dtype=np.float32)
    for k in range(P):
        o = results[k]["out"].astype(np.float32)
        out[k * NPC:(k + 1) * NPC] = o[:NPC]
        out[N + k * PRED_PC: N + (k + 1) * PRED_PC] = o[NPC:NPC + PRED_PC]
    return out


def kernel(x, edge_index, W1, b1, W2, b2, W3, b3, noise, num_missing_nodes=None,
           **_ignored):
    from concourse.bass_utils import run_bass_kernel_spmd

    nc, in_maps = _prepare(x, edge_index, W1, b1, W2, b2, W3, b3, noise,
                           num_missing_nodes)
    res = run_bass_kernel_spmd(nc, in_maps, core_ids=list(range(P)))
    global LAST_RESULTS
    LAST_RESULTS = res
    return _assemble(res.results)


if __name__ == "__main__":
    t0 = time.time()
    rng = np.random.default_rng(0)
    inputs = {
        "x": rng.standard_normal((N, C), dtype=np.float32),
        "edge_index": rng.integers(0, N, (2, E)).astype(np.int32),
        "W1": rng.standard_normal((C, C), dtype=np.float32) * 0.05,
        "b1": np.zeros(C, np.float32),
        "W2": rng.standard_normal((C, C), dtype=np.float32) * 0.05,
        "b2": np.zeros(C, np.float32),
        "W3": rng.standard_normal((C, C), dtype=np.float32) * 0.05,
        "b3": np.zeros(C, np.float32),
        "noise": rng.standard_normal((MPRED, C), dtype=np.float32),
        "num_missing_nodes": MPRED,
    }
    out = kernel(**inputs)
    print("kernel done", out.shape, time.time() - t0, "s")


# revision 19
# speedup vs baseline: 2.4686x; 1.0474x over previous
"""Trainium2 Bass kernel for a 2-layer GCN (CascadePredictionModel).

Model (per reference):
    src/dst = edge_index + self loops; deg over dst; norm_e = rsqrt(deg[src])*rsqrt(deg[dst])
    gcn(h, W, b) = segment_sum(norm * (h@W)[src], dst) + b
    h1 = relu(gcn(x,  W1, b1))
    h2 = relu(gcn(h1, W2, b2))
    pred = noise @ W3 + b3
    out = concat([h2, pred])            # [N+M, C]

Distribution strategy (8 NeuronCores, SPMD single NEFF):
  - Destination nodes are 1D-partitioned: core k owns dst rows [1250k, 1250k+1250).
  - Norm factorization: norm_e = dis[src]*dis[dst] is folded OUT of the
    selection matrices.  S holds edge MULTIPLICITIES (small exact ints) in
    fp8e4 (halving its bytes, exact); dis[src] is folded into the z-rows at
    emission (activation scale) and dis[dst] + bias at the relu via a
    rank-1 (1/dis)(x)b matmul.  Mixed fp8-lhsT x fp16-rhs matmuls are exact
    on TRN2 (verified on hw).
  - Feature matmul Z = h@W computed per-core for owned rows (weights
    replicated), scaled by dis, cast to fp16, AllGather'ed into a full
    [10000, 512] fp16 DRAM tensor.
  - Aggregation per dst tile of 128: dma_gather pulls the (per-tile deduped,
    src-sorted) source rows into SBUF as [128, nchunk, 512]; PE accumulates
    psum += S_c^T @ G_c over chunks.
  - Own-shard sources are gathered from the local pre-AllGather buffer so
    that ~1/5 of the gather volume overlaps the AllGather itself.
  - h1^T for the layer-2 matmul is built per-tile with PE transposes right
    after the layer-1 relu, so layer-2 feature matmuls pipeline behind the
    layer-1 aggregation.
  - pred rows are sharded 250/core and run inside the AllGather bubbles.

Runtime quirk this layout targets: each ExternalInput TENSOR costs ~40us
per exec on this PJRT path (host read-through), almost independent of
size — so ALL inputs are packed into a single [128, F] fp16 "blob" tensor
per core, DMA'd in 4 pipelined pieces into 4 SBUF tiles; fp8/int16/fp32
regions are accessed via bitcast views.  Gather traffic reads Internal
DRAM (device-local, fast) and is unaffected.
"""

import time
from contextlib import ExitStack

import numpy as np
from ml_dtypes import float8_e4m3fn as f8e4

N, E, C, MPRED = 10000, 160000, 512, 2000
P = 8                 # cores
NPC = N // P          # 1250 nodes per core
TPB = 128             # dst-tile width
NT = (NPC + TPB - 1) // TPB   # 10 tiles / core (last has 98 dsts)
NPAD = NT * TPB       # 1280
PRED_PC = MPRED // P  # 250 pred rows per core
KT = C // 128         # 4 contraction tiles

_prog_cache: dict[int, tuple] = {}
LAST_RESULTS = None  # BassKernelResults of the most recent run (for test.py)


# ------------------------------------------------------------- blob offsets
def _blob_layout(NCTOT):
    """f16 free-element offsets of each region; 4 DMA pieces (A, B2, B3, B4)."""
    NIDX16 = NCTOT * 8          # NIDX // 16
    # xT stored tile-major ([nt][g][128]) so an early partial load covers the
    # first feature tiles
    A = dict(W1=0, dis=KT * C)
    A["ident"] = A["dis"] + 2 * NT            # dis is f32: 2*NT f16 elems
    A["xT"] = A["ident"] + 128
    AF = A["xT"] + KT * NPAD
    B2 = dict(invdis=0, bias=NPAD + 128)
    B2["idx"] = B2["bias"] + 3 * C
    B2F = B2["idx"] + NIDX16
    B3F = NCTOT * 64                          # S fp8: NCTOT*128 bytes
    B4 = dict(nzT=0, W3=KT * 256, W2=KT * 256 + KT * C)
    B4F = B4["W2"] + KT * C
    return A, AF, B2, B2F, B3F, B4, B4F


# ---------------------------------------------------------------- host tables
def _host_tables(edge_index):
    """Build per-core gather indices + multiplicity selection matrices.

    Returns (NCHUNKS, idxs_list, S_list, dis):
      NCHUNKS      : per-dst-tile (own, other) chunk counts (max over cores)
      idxs_list[k] : [128, NIDX//16] int16  (16-partition wrap, tiled x8)
      S_list[k]    : [128, sum(NCHUNKS)*128] fp8e4 multiplicities
      dis          : [N] float32 rsqrt(deg)
    """
    ei = np.asarray(edge_index).astype(np.int64)
    src = np.concatenate([ei[0], np.arange(N, dtype=np.int64)])
    dst = np.concatenate([ei[1], np.arange(N, dtype=np.int64)])
    deg = np.bincount(dst, minlength=N).astype(np.float64)
    dis = np.where(deg > 0, 1.0 / np.sqrt(np.maximum(deg, 1.0)), 0.0)

    order = np.lexsort((src, dst))
    src_s, dst_s = src[order], dst[order]

    per_tile = []
    nown = [1] * NT
    noth = [1] * NT
    for k in range(P):
        klo, khi = k * NPC, (k + 1) * NPC
        for t in range(NT):
            lo = k * NPC + t * TPB
            hi = min(khi, lo + TPB)
            m0 = np.searchsorted(dst_s, lo)
            m1 = np.searchsorted(dst_s, hi)
            es = src_s[m0:m1]
            u = np.unique(es)
            own_mask = (u >= klo) & (u < khi)
            u_own, u_oth = u[own_mask], u[~own_mask]
            nown[t] = max(nown[t], (len(u_own) + 127) // 128)
            noth[t] = max(noth[t], (len(u_oth) + 127) // 128)
            per_tile.append((k, t, u_own, u_oth, es, dst_s[m0:m1] - lo))
    NOWN, NOTH = tuple(nown), tuple(noth)
    nch = [a + b for a, b in zip(nown, noth)]
    coff = np.concatenate([[0], np.cumsum(nch)])
    NIDX = int(coff[-1]) * 128

    idxs_list, S_list = [], []
    for k in range(P):
        idxs = np.zeros(NIDX, dtype=np.int64)
        S = np.zeros((NIDX, TPB), dtype=np.float32)
        for (kk, t, u_own, u_oth, es, dloc) in per_tile[k * NT:(k + 1) * NT]:
            base = int(coff[t]) * 128          # own group first
            obase = base + NOWN[t] * 128       # then other group
            idxs[base:base + len(u_own)] = u_own - k * NPC  # local rows in zb
            idxs[obase:obase + len(u_oth)] = u_oth          # global rows in zf
            own_e = (es >= k * NPC) & (es < (k + 1) * NPC)
            pos = np.empty(len(es), dtype=np.int64)
            pos[own_e] = base + np.searchsorted(u_own, es[own_e])
            pos[~own_e] = obase + np.searchsorted(u_oth, es[~own_e])
            np.add.at(S, (pos, dloc), 1.0)
        wrapped = np.tile(idxs.reshape(-1, 16).T, (8, 1)).astype(np.int16)
        # [NIDX, 128] -> [128(part), NCTOT, 128] -> flatten free
        S_host = np.ascontiguousarray(
            S.reshape(int(coff[-1]), 128, TPB).transpose(1, 0, 2)
        ).astype(f8e4).reshape(128, -1)
        idxs_list.append(np.ascontiguousarray(wrapped))
        S_list.append(S_host)
    return (NOWN, NOTH), idxs_list, S_list, dis.astype(np.float32)


# ---------------------------------------------------------------- device prog
def _build_program(NCHUNKS, sim1core=False, loops=1, no_cc=False, no_gather=False,
                   nqueues=4, scratch=49152):
    import concourse.bacc as bacc
    import concourse.mybir as mybir
    import concourse.tile as tile

    f16, f32, i16 = mybir.dt.float16, mybir.dt.float32, mybir.dt.int16
    f8 = mybir.dt.float8e4
    Relu = mybir.ActivationFunctionType.Relu
    Copy = mybir.ActivationFunctionType.Copy
    NOWN, NOTH = NCHUNKS
    COFF = [0]
    for a, b in zip(NOWN, NOTH):
        COFF.append(COFF[-1] + a + b)
    NCTOT = COFF[-1]
    A, AF, B2, B2F, B3F, B4, B4F = _blob_layout(NCTOT)
    TOTF = AF + B2F + B3F + B4F

    nc = bacc.Bacc(
        "TRN2", target_bir_lowering=False, debug=False,
        num_devices=(1 if sim1core else P),
        num_swdge_queues=nqueues,
        dynamic_dma_scratch_size=scratch,
    )

    blob_d = nc.dram_tensor("blob", [128, TOTF], f16, kind="ExternalInput")
    out_d = nc.dram_tensor("out", [NPC + PRED_PC, C], f16, kind="ExternalOutput")

    # z rows packed [hi fp8 | lo fp8] (1KB/row, same bytes as fp16) so the
    # aggregation can run fp8 DoubleRow matmuls (2 chunks per PE instr at
    # 0.5 cyc/row) with ~bf16 effective precision
    zb = [nc.dram_tensor(f"zb{l}", [NPC, 2 * C], f8, kind="Internal")
          for l in range(2)]
    zf = [
        nc.dram_tensor(f"zf{l}", [N, 2 * C], f8, kind="Internal",
                       addr_space=("Local" if sim1core else "Shared"))
        for l in range(2)
    ]

    with tile.TileContext(nc) as tc, ExitStack() as ctx:
        consts = ctx.enter_context(tc.tile_pool(name="consts", bufs=1))
        zpool = ctx.enter_context(tc.tile_pool(name="z", bufs=6))
        gpool = ctx.enter_context(tc.tile_pool(name="g", bufs=6))
        gown = ctx.enter_context(tc.tile_pool(name="gown", bufs=NT))
        hpool = ctx.enter_context(tc.tile_pool(name="h", bufs=3))
        opool = ctx.enter_context(tc.tile_pool(name="o", bufs=3))
        fpsum = ctx.enter_context(tc.tile_pool(name="fps", bufs=3, space="PSUM"))
        apsum = ctx.enter_context(tc.tile_pool(name="aps", bufs=3, space="PSUM"))
        tpsum = ctx.enter_context(tc.tile_pool(name="tps", bufs=2, space="PSUM"))

        tA = consts.tile([128, AF], f16, tag="tA")
        tB2 = consts.tile([128, B2F], f16, tag="tB2")
        tB3 = consts.tile([128, NCTOT, 64], f16, tag="tB3")   # S fp8 chunks
        tB4 = consts.tile([128, B4F], f16, tag="tB4")
        h1T = consts.tile([128, KT, NPAD], f16, tag="h1T")

        # pipelined input pieces, ordered by first use; tA split so the first
        # feature tiles start after ~1MB of input
        o2, o3, o4 = AF, AF + B2F, AF + B2F + B3F
        a1 = A["xT"] + 4 * KT * 128
        nc.sync.dma_start(tA[:, 0:a1], blob_d[:, 0:a1])
        nc.sync.dma_start(tA[:, a1:AF], blob_d[:, a1:AF])
        nc.sync.dma_start(tB2[:], blob_d[:, o2:o2 + B2F])
        nc.sync.dma_start(tB3[:, :, :], blob_d[:, o3:o3 + B3F])
        nc.sync.dma_start(tB4[:], blob_d[:, o4:o4 + B4F])

        # ---- views into the blob tiles
        def W1v(g):
            return tA[:, A["W1"] + g * C: A["W1"] + (g + 1) * C]

        def xTv(g, lo, w):
            # tile-major layout: [nt][g][128]
            nt = lo // 128
            o = A["xT"] + nt * KT * 128 + g * 128
            return tA[:, o:o + w]

        dis_f32 = tA[:, A["dis"]:A["dis"] + 2 * NT].bitcast(f32)  # [128, NT]
        ident = tA[:, A["ident"]:A["ident"] + 128]

        def invdisv(t):   # t in 0..NT-1; t=NT selects the ones row (pred bias)
            o = B2["invdis"] + t * 128
            return tB2[0:1, o:o + 128]

        def biasv(l):
            o = B2["bias"] + l * C
            return tB2[0:1, o:o + C]

        def idxv(e0, e1):  # i16 element range
            return tB2[:, B2["idx"] + e0:B2["idx"] + e1].bitcast(i16)

        def Sv(c):        # chunk c -> [128, 128] fp8
            return tB3[:, c, :].bitcast(f8)

        def SvP(c):       # chunk pair (c, c+1) -> [128, 2, 128] fp8
            return tB3[:, c:c + 2, :].bitcast(f8)

        def nzTv(g, mt):
            o = B4["nzT"] + g * 256 + mt * 128
            return tB4[:, o:o + 128]

        def W2v(g):
            return tB4[:, B4["W2"] + g * C: B4["W2"] + (g + 1) * C]

        def W3v(g):
            return tB4[:, B4["W3"] + g * C: B4["W3"] + (g + 1) * C]

        def emit_z(ps, t, zb_d):
            # z = dis*psum split into fp8 hi + lo, packed [hi | lo] per row
            zhi = zpool.tile([128, C], f8, tag="zhi")
            nc.scalar.activation(zhi[:], ps[:], Copy, scale=dis_f32[:, t:t + 1])
            z16 = zpool.tile([128, C], f16, tag="z16")
            nc.scalar.activation(z16[:], ps[:], Copy, scale=dis_f32[:, t:t + 1])
            zlo = zpool.tile([128, C], f8, tag="zlo")
            nc.vector.tensor_tensor(zlo[:], z16[:], zhi[:],
                                    bacc.mybir.AluOpType.subtract)
            w = NPC - t * 128 if t == NT - 1 else 128
            nc.sync.dma_start(zb_d[t * 128: t * 128 + w, 0:C], zhi[:w, :])
            nc.sync.dma_start(zb_d[t * 128: t * 128 + w, C:2 * C], zlo[:w, :])

        def feature_layer(lhsT_v, Wv, zb_d):
            # zb = dis (.) (h @ W)   (src-side norm folded in at emission)
            for nt in range(NT):
                ps = fpsum.tile([128, C], f32, tag="fps")
                for g in range(KT):
                    nc.tensor.matmul(
                        ps[:], lhsT_v(g, nt * 128, 128), Wv(g),
                        start=(g == 0), stop=(g == KT - 1),
                    )
                emit_z(ps, nt, zb_d)

        def tile_groups(nch):
            # split point even so DoubleRow pairs stay aligned in both groups
            ha = min((((nch + 1) // 2) + 1) // 2 * 2, 8)
            return [(0, ha), (ha, nch)] if nch > ha else [(0, nch)]

        _qn = [0]

        def gather_group(src_d, t, c0, c1, pool=None, tag="g"):
            """One dma_gather of chunks [c0,c1) of tile t from src_d."""
            qn = _qn[0] % nqueues
            _qn[0] += 1
            nch = c1 - c0
            G = (pool or gpool).tile([128, nch, 2 * C], f8, tag=tag)
            if no_gather:
                nc.vector.memset(G[:, 0, 0:16].bitcast(f16), 0.0)
            else:
                nc.gpsimd.dma_gather(
                    G[:],
                    src_d[:],
                    idxv(COFF[t] * 8 + c0 * 8, COFF[t] * 8 + c1 * 8),
                    nch * 128,
                    nch * 128,
                    2 * C,
                    single_packet=(nch * 128 <= 1024),
                    queue_num=qn,
                )
            return G

        def agg_own_gathers(zb_d):
            """Own-shard gathers (from the local pre-AllGather buffer) — these
            only need zb, so they run during the AllGather wait."""
            return [gather_group(zb_d, t, 0, NOWN[t], pool=gown, tag="go")
                    for t in range(NT)]

        DR = bacc.mybir.MatmulPerfMode.DoubleRow

        def agg_mms(ps, G, sbase, c0, c1, start):
            """Accumulate chunks [c0,c1) of G (local coords) into ps with S
            chunks sbase+c0..: DoubleRow pairs + odd-tail singles, hi then lo
            halves of the packed fp8 rows."""
            first = [start]
            c = c0
            while c < c1:
                if c + 1 < c1:
                    lhsT = SvP(sbase + c)
                    for h in range(2):
                        nc.tensor.matmul(
                            ps[:], lhsT,
                            G[:, c - c0:c - c0 + 2, h * C:(h + 1) * C],
                            start=first[0], stop=False, perf_mode=DR,
                        )
                        first[0] = False
                    c += 2
                else:
                    lhsT = Sv(sbase + c)
                    for h in range(2):
                        nc.tensor.matmul(
                            ps[:], lhsT, G[:, c - c0, h * C:(h + 1) * C],
                            start=first[0], stop=False,
                        )
                        first[0] = False
                    c += 1

        def agg_layer(lidx, zf_d, own_G, emit_out):
            # psum = sum_s mult * (dis.z)_s ; + rank-1 (1/dis)(x)b
            for t in range(NT):
                ps = apsum.tile([128, C], f32, tag="aps")
                agg_mms(ps, own_G[t], COFF[t], 0, NOWN[t], True)
                for (c0, c1) in tile_groups(NOTH[t]):
                    G = gather_group(zf_d, t, NOWN[t] + c0, NOWN[t] + c1)
                    agg_mms(ps, G, COFF[t] + NOWN[t], c0, c1, False)
                nc.tensor.matmul(
                    ps[:], invdisv(t), biasv(lidx), start=False, stop=True,
                )
                emit_out(t, ps)

        rg = [list(range(P))]

        def allgather(l):
            if no_cc or sim1core:
                nc.sync.dma_start(zf[l][:NPC, :], zb[l][:])
            else:
                nc.gpsimd.collective_compute(
                    "AllGather",
                    bacc.mybir.AluOpType.bypass,
                    replica_groups=rg,
                    ins=[zb[l][:]],
                    outs=[zf[l][:]],
                )

        # pred = noise @ W3 + b3 (no relu), 250 rows/core — one tile emitted in
        # each AllGather bubble so the PE has work while waiting.
        def pred_tile(mt):
            ps = fpsum.tile([128, C], f32, tag="fps")
            for g in range(KT):
                nc.tensor.matmul(
                    ps[:], nzTv(g, mt), W3v(g), start=(g == 0), stop=False,
                )
            nc.tensor.matmul(
                ps[:], invdisv(NT), biasv(2), start=False, stop=True,
            )
            ot = opool.tile([128, C], f16, tag="o")
            nc.scalar.activation(ot[:], ps[:], Copy)
            w = min(128, PRED_PC - mt * 128)
            nc.sync.dma_start(
                out_d[NPC + mt * 128: NPC + mt * 128 + w, :], ot[:w, :]
            )

        def l1_out(t, ps):
            # h1 = relu(dis*psum + b1); PE-transpose into h1T and IMMEDIATELY
            # run the layer-2 feature matmul for this node tile, so zb1 is
            # complete right after the layer-1 aggregation and AllGather-2 can
            # start ~20us earlier (kills the DMA idle hole between layers).
            ht = hpool.tile([128, C], f16, tag="h")
            nc.scalar.activation(ht[:], ps[:], Relu, scale=dis_f32[:, t:t + 1])
            for g in range(KT):
                pt = tpsum.tile([128, 128], f16, tag="tps")
                nc.tensor.transpose(pt[:], ht[:, g * 128:(g + 1) * 128], ident)
                nc.vector.tensor_copy(h1T[:, g, t * 128:(t + 1) * 128], pt[:])
            ps2 = fpsum.tile([128, C], f32, tag="fps")
            for g in range(KT):
                nc.tensor.matmul(
                    ps2[:], h1T[:, g, t * 128:(t + 1) * 128], W2v(g),
                    start=(g == 0), stop=(g == KT - 1),
                )
            emit_z(ps2, t, zb[1])

        def l2_out(t, ps):
            ot = opool.tile([128, C], f16, tag="o")
            nc.scalar.activation(ot[:], ps[:], Relu, scale=dis_f32[:, t:t + 1])
            w = NPC - t * 128 if t == NT - 1 else 128
            nc.sync.dma_start(out_d[t * 128: t * 128 + w, :], ot[:w, :])

        for _rep in range(loops):
            # ---- layer 1 (l1_out also emits the fused layer-2 features)
            feature_layer(xTv, W1v, zb[0])
            own1 = agg_own_gathers(zb[0])
            pred_tile(0)
            allgather(0)
            agg_layer(0, zf[0], own1, l1_out)
            # ---- layer 2
            own2 = agg_own_gathers(zb[1])
            pred_tile(1)
            allgather(1)
            agg_layer(1, zf[1], own2, l2_out)

    nc.compile()
    return nc


def _get_program(NCHUNKS):
    if NCHUNKS not in _prog_cache:
        _prog_cache[NCHUNKS] = _build_program(NCHUNKS)
    return _prog_cache[NCHUNKS]


# ---------------------------------------------------------------- entry point
def _prepare(x, edge_index, W1, b1, W2, b2, W3, b3, noise, num_missing_nodes=None,
             **_ignored):
    """Host preprocessing: returns (nc, in_maps)."""
    x = np.asarray(x, dtype=np.float32)
    W1 = np.asarray(W1, dtype=np.float32)
    W2 = np.asarray(W2, dtype=np.float32)
    W3 = np.asarray(W3, dtype=np.float32)
    b1 = np.asarray(b1, dtype=np.float32)
    b2 = np.asarray(b2, dtype=np.float32)
    b3 = np.asarray(b3, dtype=np.float32)
    noise = np.asarray(noise, dtype=np.float32)

    NCHUNKS, idxs_list, S_list, dis = _host_tables(edge_index)
    nc = _get_program(NCHUNKS)
    NOWN, NOTH = NCHUNKS
    NCTOT = sum(NOWN) + sum(NOTH)
    A, AF, B2, B2F, B3F, B4, B4F = _blob_layout(NCTOT)
    TOTF = AF + B2F + B3F + B4F
    o2, o3, o4 = AF, AF + B2F, AF + B2F + B3F

    def wtiles(W):
        # [512, 512] -> [128, KT*512] fp16
        return np.ascontiguousarray(
            W.reshape(KT, 128, C).transpose(1, 0, 2)
        ).astype(np.float16).reshape(128, -1)

    biasr = np.concatenate([b1, b2, b3]).astype(np.float16)      # [3*C]
    identity = np.eye(128, dtype=np.float16)
    w1t, w2t, w3t = wtiles(W1), wtiles(W2), wtiles(W3)

    in_maps = []
    for k in range(P):
        blob = np.zeros((128, TOTF), dtype=np.float16)
        # ---- A piece
        blob[:, A["W1"]:A["W1"] + KT * C] = w1t
        xs = np.zeros((NPAD, C), dtype=np.float16)
        xs[:NPC] = x[k * NPC:(k + 1) * NPC].astype(np.float16)
        # [128(part=c in g), nt, g, 128(node)] tile-major free layout
        xT = np.ascontiguousarray(
            xs.T.reshape(KT, 128, NT, 128).transpose(1, 2, 0, 3)
        ).reshape(128, -1)
        blob[:, A["xT"]:A["xT"] + KT * NPAD] = xT
        dk = np.ones(NPAD, dtype=np.float32)
        dk[:NPC] = dis[k * NPC:(k + 1) * NPC]
        dk = np.maximum(dk, 1e-6)
        dis_t = np.ascontiguousarray(dk.reshape(NT, 128).T)      # [128, NT] f32
        blob[:, A["dis"]:A["dis"] + 2 * NT] = dis_t.view(np.float16)
        blob[:, A["ident"]:A["ident"] + 128] = identity
        # ---- B2 piece (invdis | bias | idx), invdis+bias replicated over rows
        invd = np.ones(NPAD + 128, dtype=np.float32)
        invd[:NPAD] = 1.0 / dk
        blob[:, o2 + B2["invdis"]:o2 + B2["invdis"] + NPAD + 128] = \
            invd.astype(np.float16)[None, :]
        blob[:, o2 + B2["bias"]:o2 + B2["bias"] + 3 * C] = biasr[None, :]
        blob[:, o2 + B2["idx"]:o2 + B2["idx"] + NCTOT * 8] = \
            idxs_list[k].view(np.float16)
        # ---- B3 piece (S fp8)
        blob[:, o3:o3 + B3F] = S_list[k].view(np.float16)
        # ---- B4 piece (nzT | W3 | W2)
        nz = np.zeros((256, C), dtype=np.float16)
        nz[:PRED_PC] = noise[k * PRED_PC:(k + 1) * PRED_PC].astype(np.float16)
        nzT = np.ascontiguousarray(
            nz.T.reshape(KT, 128, 256).transpose(1, 0, 2)).reshape(128, -1)
        blob[:, o4 + B4["nzT"]:o4 + B4["nzT"] + KT * 256] = nzT
        blob[:, o4 + B4["W3"]:o4 + B4["W3"] + KT * C] = w3t
        blob[:, o4 + B4["W2"]:o4 + B4["W2"] + KT * C] = w2t
        in_maps.append({"blob": blob})

    return nc, in_maps


def _assemble(results):
    out = np.empty((N + MPRED, C), dtype=np.float32)
    for k in range(P):
        o = results[k]["out"].astype(np.float32)
        out[k * NPC:(k + 1) * NPC] = o[:NPC]
        out[N + k * PRED_PC: N + (k + 1) * PRED_PC] = o[NPC:NPC + PRED_PC]
    return out


def kernel(x, edge_index, W1, b1, W2, b2, W3, b3, noise, num_missing_nodes=None,
           **_ignored):
    from concourse.bass_utils import run_bass_kernel_spmd

    nc, in_maps = _prepare(x, edge_index, W1, b1, W2, b2, W3, b3, noise,
                           num_missing_nodes)
    res = run_bass_kernel_spmd(nc, in_maps, core_ids=list(range(P)))
    global LAST_RESULTS
    LAST_RESULTS = res
    return _assemble(res.results)


if __name__ == "__main__":
    t0 = time.time()
    rng = np.random.default_rng(0)
    inputs = {
        "x": rng.standard_normal((N, C), dtype=np.float32),
        "edge_index": rng.integers(0, N, (2, E)).astype(np.int32),
        "W1": rng.standard_normal((C, C), dtype=np.float32) * 0.05,
        "b1": np.zeros(C, np.float32),
        "W2": rng.standard_normal((C, C), dtype=np.float32) * 0.05,
        "b2": np.zeros(C, np.float32),
        "W3": rng.standard_normal((C, C), dtype=np.float32) * 0.05,
        "b3": np.zeros(C, np.float32),
        "noise": rng.standard_normal((MPRED, C), dtype=np.float32),
        "num_missing_nodes": MPRED,
    }
    out = kernel(**inputs)
    print("kernel done", out.shape, time.time() - t0, "s")
